# revision 27
# baseline (speedup 1.0000x reference)
"""Trainium2 Bass kernel for nn_ContrastiveLoss2 (SimCLR NT-Xent loss).

Math (matches the jax reference):
    z  = concat([z_augment, z_orig])                       # [N=8192, D=256]
    zn = z / max(||z||, eps)                               # row L2 normalize
    S  = zn @ zn.T                                         # cosine sim [N, N]
    loss_i = -2 S[i, i+-B] + log( sum_{j != i} exp(2 S[i,j]) )
    out = mean_i loss_i                                    # tau = 0.5

Identity: denominator_i = sum_j exp(2 S_ij) - e^2 (S_ii == 1).

SYMMETRIC distribution (v2): S is symmetric, so each unordered pair is
computed once fleet-wide.  Core c (rows rolled so its own 1024 rows sit
at [0:1024)) computes the S block [5120 j-rows x 1024 own columns]:
  - j-tiles 0..31  (cores c..c+3):   full weight
  - j-tiles 32..39 (core c+4):       half weight, exp(2S - ln2); the
    partner core c+4 computes the transposed block also at half weight,
    so every pair still sums to 1.
Per-core outputs (denominators are assembled on the HOST):
  - csrow [2, 512]: column sums of exp over the block's 5120 j-rows
    (PE fp8 ones-matmul into PSUM) -> partial denominators for the
    core's own 1024 rows.  Row 0 holds cols 0:512, row 1 cols 512:1024.
  - rs [128, 32]: free-dim row sums of the exp tiles 8..39 -> partial
    denominators for rows owned by cores c+1..c+4.  ACT tiles get these
    free via the activation accumulator; Schraudolph tiles use a
    tensor_scalar+accum on DVE/Pool over the fp8 exp tile.
  - pose [128, 8]: the positive entries, read from the exp tiles
    (tiles 32..39 diagonal) by a Pool masked multiply+accum; the host
    inverts exp (or the Schraudolph bit pattern) to recover 2 S_pos.
Host: r_i = own colsum + 4 partner rowsums; loss = (sum ln(r_i - e^2)
- sum 2 S_pos) / N.

Engines: exp tiles split ACT (activation Exp, per-partition scale
inv_j) / DVE (Schraudolph: int8 = S*a_j + b bit-pattern IS fp8 exp);
norm prep: own rows via DVE tensor_tensor_reduce, j-rows via Pool
square + add-tree; PE: S matmuls + fp8 DoubleRow colsum accumulation.
"""

import sys

import numpy as np

try:
    import concourse  # noqa: F401
except ImportError:  # pragma: no cover
    sys.path.insert(0, "/opt/trn_rl_repo")

N_CORES = 8
N = 8192          # total rows (2B)
D = 256           # feature dim
B = 4096          # batch (positive offset)
P = 128           # SBUF partitions
NT = 40           # j-tiles per core (5/8 of 64)
NJ = NT * P       # 5120 j rows per core
RPC = 1024        # own columns per core
NI = RPC // P     # 8 own col-tiles
TAU = 0.5
E2 = float(np.exp(2.0))
LN2 = float(np.log(2.0))
A_EXP = 8.0 / LN2      # rhs carries the factor 2 -> a = inv * 8/ln2
SIGMA = 0.0435
# real-HW fp32->int8 convert rounds to nearest (the simulator truncates);
# calibrate for hardware, the graded correctness path
B_EXP = 56.0 - 8.0 * SIGMA

# per-tile exp engine assignment: A(CT) / D(VE Schraudolph); tiles 0..3
# run as ACT half-tiles during the fill (before the second rhs half is
# ready), the rest alternate starting with DVE
ASSIGN = ['A', 'A', 'A', 'A'] + ['D', 'A'] * 18

# Pool prep chunks over j-tiles 8..39 (own tiles 0..7 use DVE ttr)
CHUNKS = [(8, 16), (16, 24), (24, 32), (32, 40)]


def _kernel_body(ctx, tc, csrow_ap, rs_ap, pose_ap, zn_ap, zt_ap):
    from concourse import mybir
    from concourse.masks import make_identity

    nc = tc.nc
    f32 = mybir.dt.float32
    bf16 = mybir.dt.bfloat16
    fp8 = mybir.dt.float8e4
    i8 = mybir.dt.int8
    Fn = mybir.ActivationFunctionType
    Op = mybir.AluOpType
    DR = mybir.MatmulPerfMode.DoubleRow

    p_const = ctx.enter_context(tc.tile_pool(name="const", bufs=1))
    p_z = ctx.enter_context(tc.tile_pool(name="z", bufs=1))
    p_sq = ctx.enter_context(tc.tile_pool(name="sq", bufs=1))
    p_tree = ctx.enter_context(tc.tile_pool(name="tree", bufs=1))
    p_stats = ctx.enter_context(tc.tile_pool(name="stats", bufs=1))
    p_ex = ctx.enter_context(tc.tile_pool(name="ex", bufs=6))
    p_junk = ctx.enter_context(tc.tile_pool(name="junk", bufs=2))
    p_s = ctx.enter_context(tc.tile_pool(name="s", bufs=2, space="PSUM"))
    p_sh = ctx.enter_context(tc.tile_pool(name="sh", bufs=2, space="PSUM"))
    p_cs = ctx.enter_context(tc.tile_pool(name="cs", bufs=1, space="PSUM"))

    znat = p_z.tile([P, NT, D], bf16, tag="znat", name="znat")
    zT = p_z.tile([P, 2, NJ], fp8, tag="zT", name="zT")
    sq = p_sq.tile([P, 8, D], bf16, tag="sq")
    # tree levels for Pool chunks: widths 128 ... 2 bf16, final add -> f32
    tl = [p_tree.tile([P, 8, D // (2 << k)], bf16, tag=f"tl{k}", name=f"tl{k}")
          for k in range(7)]
    sqj = p_sq.tile([P, D], bf16, tag="sqj")  # ttr junk out (own prep)
    ss = p_stats.tile([P, NT], f32, tag="ss")
    lns = p_stats.tile([P, NT], f32, tag="lns")
    inv = p_stats.tile([P, NT], f32, tag="inv")
    a_col = p_stats.tile([P, NT], f32, tag="a_col")
    inv2own = p_stats.tile([P, NI], f32, tag="inv2own")
    ln2_c = p_const.tile([P, 1], f32, tag="ln2c")
    nln2_c = p_const.tile([P, 1], f32, tag="nln2c")
    masked = p_stats.tile([P, RPC], bf16, tag="masked")
    rhs = p_z.tile([P, 2, RPC], fp8, tag="rhs", name="rhs")
    ones_bf = p_const.tile([P, P], bf16, tag="onesbf")
    ones = p_const.tile([P, 2, P], fp8, tag="ones")
    ident = p_const.tile([P, P], bf16, tag="ident")
    rs = p_stats.tile([P, 32], f32, tag="rs")
    posE = p_stats.tile([P, NI], f32, tag="posE")
    csrow = p_stats.tile([1, RPC], f32, tag="csrow")

    from concourse import library_config
    nc.gpsimd.load_library(library_config.proxy)

    # input DMAs, spread across queues; zT0 on ACT, zT1 on DVE so the SP
    # queue's serial issue stream starts with the prep-critical zn pieces
    def load_zn(eng, t0, t1):
        eng.dma_start(out=znat[:, t0:t1, :], in_=zn_ap[:, t0 * D:t1 * D]
                      .rearrange("p (t c) -> p t c", c=D))

    def load_zt(eng, k):
        eng.dma_start(out=zT[:, :, k * 1024:(k + 1) * 1024],
                      in_=zt_ap[:, :, k * 1024:(k + 1) * 1024]
                      .rearrange("h p j -> p h j"))

    load_zt(nc.scalar, 0)
    load_zn(nc.sync, 0, 4)
    load_zn(nc.sync, 4, 8)
    load_zn(nc.sync, 8, 16)
    load_zt(nc.sync, 1)
    load_zn(nc.sync, 16, 24)
    load_zt(nc.sync, 2)
    load_zn(nc.sync, 24, 32)
    load_zt(nc.sync, 3)
    load_zn(nc.sync, 32, 40)
    load_zt(nc.sync, 4)

    # consts (DVE memsets are cheap; ident is built on Pool)
    nc.vector.memset(ones_bf[:], 1.0)
    nc.vector.memset(ones[:], 1.0)
    nc.vector.memset(ln2_c[:], LN2)
    nc.vector.memset(nln2_c[:], -LN2)
    make_identity(nc, ident[:])
    junkw = p_const.tile([P, 512], bf16, tag="junkw")
    nc.gpsimd.memset(junkw[:], 0.0)

    # preload the Ln/Exp activation table off the critical path
    warm = p_const.tile([1, 1], f32, tag="warm")
    warm_o = p_const.tile([1, 1], f32, tag="warmo")
    nc.gpsimd.memset(warm[:], 1.0)
    nc.scalar.activation(warm_o[:], warm[:], Fn.Ln)

    cs = p_cs.tile([P, RPC], f32)

    # PE p-state warm-up: dummy matmuls into the (not yet used) cs banks
    # keep the Tensor engine continuously busy through the fill so the
    # first real matmuls run at full clock instead of 0.65 GHz
    for _ in range(10):
        nc.tensor.matmul(cs[:, 0:512], lhsT=ones_bf[:], rhs=junkw[:],
                         start=True, stop=True)

    # --- own-row prep: all 8 tiles on DVE (fused square+reduce via
    # stt+accum, the HW-safe tensor_tensor_reduce replacement), the ACT
    # ln/exp chain per 4-tile half overlapped with the second stt half ---
    for t in range(0, 8):
        nc.vector.scalar_tensor_tensor(
            sqj[:], znat[:, t, :], 1.0, znat[:, t, :],
            op0=Op.mult, op1=Op.mult, accum_out=ss[:, t:t + 1])

    # single chain with true dependencies so the scheduler cannot
    # interleave it into (and thereby stretch) the stt stream
    nc.scalar.activation(lns[:, 0:8], ss[:, 0:8], Fn.Ln)
    nc.scalar.activation(inv[:, 0:8], lns[:, 0:8], Fn.Exp, scale=-0.5)
    # 2/||z|| for the own rows (rhs carries the factor 2)
    nc.scalar.activation(inv2own[:], lns[:, 0:8], Fn.Exp,
                         scale=-0.5, bias=ln2_c[:])
    # replicate inv2own across partitions: identity-mask (broadcast
    # views, DVE 2x) + bf16 ones-matmul column sums into the cs PSUM
    # banks (free until the first colsum accumulation)
    nc.vector.tensor_tensor(
        masked[:].rearrange("p (t q) -> p t q", q=P),
        inv2own[:].rearrange("p (t o) -> p t o", o=1)
        .broadcast_to((P, NI, P)),
        ident[:].rearrange("p (o q) -> p o q", o=1)
        .broadcast_to((P, NI, P)),
        op=Op.mult)
    for h in range(2):
        nc.tensor.matmul(cs[:, h * 512:(h + 1) * 512], lhsT=ones_bf[:],
                         rhs=masked[:, h * 512:(h + 1) * 512],
                         start=True, stop=True)
        nc.vector.tensor_tensor(
            rhs[:, :, h * 512:(h + 1) * 512],
            zT[:, :, h * 512:(h + 1) * 512],
            cs[:, h * 512:(h + 1) * 512].rearrange("p (o c) -> p o c", o=1)
            .broadcast_to((P, 2, 512)),
            op=Op.mult)

    # --- Pool prep for j-tile chunks 8..39: square + binary add-tree.
    # All chunks share the same sq/tl scratch slots, so WAR/WAW deps keep
    # the chunks strictly in order (the scheduler otherwise hoists a later
    # chunk's big square in front of an earlier chunk's tree tail). ---
    def chunk_sq(t0, t1):
        nc.gpsimd.tensor_tensor(sq[:, 0:t1 - t0, :], znat[:, t0:t1, :],
                                znat[:, t0:t1, :], op=Op.mult)
        src = sq[:, 0:t1 - t0, :].rearrange(
            "p t (two c) -> p t two c", two=2)
        nc.gpsimd.tensor_tensor(tl[0][:, 0:t1 - t0, :], src[:, :, 0, :],
                                src[:, :, 1, :], op=Op.add)

    def chunk_tree(t0, t1, k0, k1):
        for k in range(k0, k1):
            s2 = tl[k][:, 0:t1 - t0, :].rearrange(
                "p t (two c) -> p t two c", two=2)
            nc.gpsimd.tensor_tensor(tl[k + 1][:, 0:t1 - t0, :],
                                    s2[:, :, 0, :], s2[:, :, 1, :], op=Op.add)

    def chunk_fin(t0, t1):
        s2 = tl[6][:, 0:t1 - t0, :]
        nc.gpsimd.tensor_tensor(
            ss[:, t0:t1].rearrange("p (t o) -> p t o", o=1),
            s2[:, :, 0:1], s2[:, :, 1:2], op=Op.add)
        nc.scalar.activation(lns[:, t0:t1], ss[:, t0:t1], Fn.Ln)
        nc.scalar.activation(inv[:, t0:t1], lns[:, t0:t1], Fn.Exp, scale=-0.5)
        nc.vector.tensor_scalar(a_col[:, t0:t1], inv[:, t0:t1], A_EXP, None,
                                op0=Op.mult)

    # a_col for own tiles (DVE Schraudolph scale)
    nc.vector.tensor_scalar(a_col[:, 0:8], inv[:, 0:8], A_EXP, None,
                            op0=Op.mult)

    ex_state = {}
    pend_cs = []

    def do_tile(t):
        u, slot = divmod(t, 2)
        if slot == 0:
            ex = p_ex.tile([P, 2, RPC], fp8, tag="ex", name="ex")
            ex_state['ex'] = ex
        else:
            ex = ex_state['ex']
        if t < 4:
            # fill phase: run as two half-width tiles in dedicated 1-bank
            # PSUM tiles so the c0 half can be exp'd before the second rhs
            # half exists (PSUM deps are tile-granular)
            for c in range(2):
                sh = p_sh.tile([P, 512], f32, tag="sh", name="sh")
                nc.tensor.matmul(
                    sh[:], lhsT=zT[:, :, t * P:(t + 1) * P],
                    rhs=rhs[:, :, c * 512:(c + 1) * 512],
                    start=True, stop=True, perf_mode=DR)
                nc.scalar.activation(ex[:, slot, c * 512:(c + 1) * 512],
                                     sh[:], Fn.Exp, scale=inv[:, t:t + 1])
            if slot == 1:
                pend_cs.append((u, ex))
            return
        s_ps = p_s.tile([P, RPC], f32, tag="s", name="s_ps")
        for c in range(2):
            nc.tensor.matmul(
                s_ps[:, c * 512:(c + 1) * 512],
                lhsT=zT[:, :, t * P:(t + 1) * P],
                rhs=rhs[:, :, c * 512:(c + 1) * 512],
                start=True, stop=True, perf_mode=DR)
        half = t >= 32
        if ASSIGN[t] == 'A':
            kw = {}
            if t >= 8:
                kw['accum_out'] = rs[:, t - 8:t - 7]
            if half:
                kw['bias'] = nln2_c[:]
            nc.scalar.activation(ex[:, slot, :], s_ps[:], Fn.Exp,
                                 scale=inv[:, t:t + 1], **kw)
        else:
            b = B_EXP - (8.0 if half else 0.0)
            nc.vector.tensor_scalar(ex[:, slot, :].bitcast(i8), s_ps[:],
                                    a_col[:, t:t + 1], b,
                                    op0=Op.mult, op1=Op.add)
            if t >= 8:
                # rowsum: Pool folds the fp8 tile 1024 -> 64 with 4 TT
                # adds (no Pool tensor_scalar on real HW); DVE finishes
                # with a 2x tensor_scalar + accumulate
                exs = ex[:, slot, :]
                fold = p_junk.tile([P, 960], bf16, tag="fold", name="fold")
                nc.gpsimd.tensor_tensor(fold[:, 0:512], exs[0:P, 0:512],
                                        exs[0:P, 512:1024], op=Op.add)
                nc.gpsimd.tensor_tensor(fold[:, 512:768], fold[:, 0:256],
                                        fold[:, 256:512], op=Op.add)
                nc.gpsimd.tensor_tensor(fold[:, 768:896], fold[:, 512:640],
                                        fold[:, 640:768], op=Op.add)
                nc.gpsimd.tensor_tensor(fold[:, 896:960], fold[:, 768:832],
                                        fold[:, 832:896], op=Op.add)
                junk = p_junk.tile([P, 64], bf16, tag="junk", name="junk")
                nc.vector.tensor_scalar(junk[:], fold[:, 896:960], 1.0, None,
                                        op0=Op.mult, op1=Op.add,
                                        accum_out=rs[:, t - 8:t - 7])
        if half:
            # positive entries: diagonal of col-block (t-32) of the exp
            # tile, via DVE masked multiply + accumulate (all-SBUF 2x)
            k = t - 32
            junkm = p_junk.tile([P, P], bf16, tag="junkm", name="junkm")
            nc.vector.scalar_tensor_tensor(
                junkm[:], ex[:, slot, k * P:(k + 1) * P], 1.0, ident[:],
                op0=Op.mult, op1=Op.mult, accum_out=posE[:, k:k + 1])
        if slot == 1:
            pend_cs.append((u, ex))
        # defer colsum matmuls so a lagging exp pair can't stall the S
        # matmuls behind it in PE's in-order queue (taper the defer near
        # the end so the final flush isn't a burst after the last exp)
        defer = 7 if t < 32 else 3
        while pend_cs and (pend_cs[0][0] * 2 + defer <= t or t == NT - 1):
            uu, exx = pend_cs.pop(0)
            for c in range(2):
                nc.tensor.matmul(
                    cs[:, c * 512:(c + 1) * 512],
                    lhsT=ones[:], rhs=exx[:, :, c * 512:(c + 1) * 512],
                    start=(uu == 0), stop=(uu == NT // 2 - 1), perf_mode=DR)

    # interleave Pool chunk prep into the tile stream so chunk c's ops
    # don't block the Pool rowsum/pos ops of earlier tiles
    PREP_AT = {
        0: lambda: chunk_sq(8, 16),
        1: lambda: chunk_tree(8, 16, 0, 3),
        2: lambda: chunk_tree(8, 16, 3, 6),
        3: lambda: chunk_fin(8, 16),
        4: lambda: chunk_sq(16, 24),
        6: lambda: chunk_tree(16, 24, 0, 3),
        8: lambda: chunk_tree(16, 24, 3, 6),
        10: lambda: chunk_fin(16, 24),
        12: lambda: chunk_sq(24, 32),
        14: lambda: chunk_tree(24, 32, 0, 3),
        16: lambda: chunk_tree(24, 32, 3, 6),
        18: lambda: chunk_fin(24, 32),
        20: lambda: chunk_sq(32, 40),
        22: lambda: chunk_tree(32, 40, 0, 3),
        24: lambda: chunk_tree(32, 40, 3, 6),
        26: lambda: chunk_fin(32, 40),
    }
    for t in range(NT):
        do_tile(t)
        if t in PREP_AT:
            PREP_AT[t]()

    # tail: cs row readout split DVE/ACT (the 128 cs rows are identical),
    # then the two csrow halves DMA'd from separate queues in parallel
    nc.vector.tensor_scalar(csrow[:, 0:512], cs[0:1, 0:512], 1.0, None,
                            op0=Op.mult)
    nc.scalar.activation(csrow[:, 512:1024], cs[0:1, 512:1024], Fn.Copy)
    nc.sync.dma_start(out=pose_ap, in_=posE[:])
    nc.sync.dma_start(out=csrow_ap[:, 0:512], in_=csrow[:, 0:512])
    nc.scalar.dma_start(out=csrow_ap[:, 512:1024], in_=csrow[:, 512:1024])
    nc.sync.dma_start(out=rs_ap, in_=rs[:])


def build_nc():
    """Build (once) the Bass module shared by all 8 cores."""
    from contextlib import ExitStack

    from concourse import bacc, mybir
    import concourse.tile as tile

    nc = bacc.Bacc("TRN2", target_bir_lowering=False, debug=False)
    fp8 = mybir.dt.float8e4
    zn = nc.dram_tensor("zn", [P, NT * D], mybir.dt.bfloat16,
                        kind="ExternalInput").ap()
    zt = nc.dram_tensor("zt", [2, P, NJ], fp8, kind="ExternalInput").ap()
    csrow = nc.dram_tensor("csrow", [1, RPC], mybir.dt.float32,
                           kind="ExternalOutput").ap()
    rs = nc.dram_tensor("rs", [P, 32], mybir.dt.float32,
                        kind="ExternalOutput").ap()
    pose = nc.dram_tensor("pose", [P, NI], mybir.dt.float32,
                          kind="ExternalOutput").ap()
    with tile.TileContext(nc) as tc:
        with ExitStack() as ctx:
            _kernel_body(ctx, tc, csrow, rs, pose, zn, zt)
    return nc


_NC = None


def _get_nc(finalized=True):
    global _NC
    if _NC is None:
        _NC = build_nc()
    if finalized and not _NC.is_finalized():
        _NC.finalize()
    return _NC


def make_in_maps(z_orig, z_augment):
    from concourse import mybir

    f8np = mybir.dt.np(mybir.dt.float8e4)
    bfnp = mybir.dt.np(mybir.dt.bfloat16)
    z = np.ascontiguousarray(
        np.concatenate([np.asarray(z_augment, dtype=np.float32),
                        np.asarray(z_orig, dtype=np.float32)], axis=0))
    maps = []
    for c in range(N_CORES):
        zr = np.roll(z, -RPC * c, axis=0)[:NJ]
        zf8 = zr.astype(f8np)
        zbf = zr.astype(bfnp)
        # natural, pre-swizzled: zn[p, t*256 + c] = z[t*128+p, c]
        znat = np.ascontiguousarray(
            zbf.reshape(NT, P, D).transpose(1, 0, 2).reshape(P, NT * D))
        # transposed: zt[h, p, j] = z[j, 128h + p]
        zt = np.ascontiguousarray(zf8.T.reshape(2, P, NJ))
        maps.append({"zn": znat, "zt": zt})
    return maps


def reduce_outputs(results):
    """Host assembly: denominators from colsum + rowsums, ln, positives."""
    r = np.zeros(N, dtype=np.float64)
    pos_total = 0.0
    for c, res in enumerate(results):
        base = RPC * c
        cs = np.asarray(res["csrow"], dtype=np.float64).reshape(RPC)
        idx = (base + np.arange(RPC)) % N
        r[idx] += cs
        rsv = np.asarray(res["rs"], dtype=np.float64)  # [128, 32]
        for t in range(8, NT):
            jdx = (base + P * t + np.arange(P)) % N
            r[jdx] += rsv[:, t - 8]
        # positives: decode the exp-tile values back to 2 S_cos
        pe = np.asarray(res["pose"], dtype=np.float32)  # [128, 8]
        for k in range(NI):
            t = 32 + k
            v = pe[:, k].astype(np.float64)
            if ASSIGN[t] == 'A':
                # v = fp8(exp(2S - ln2)) -> 2S = ln(2 v)
                twos = np.log(np.maximum(2.0 * v, 1e-30))
            else:
                # v = fp8-bit-pattern Schraudolph: bits = 2S*8/ln2 + B - 8
                from concourse import mybir
                f8np = mybir.dt.np(mybir.dt.float8e4)
                bits = pe[:, k].astype(f8np).view(np.uint8).astype(np.float64)
                twos = (bits - (B_EXP - 8.0)) * LN2 / 8.0
            pos_total += float(twos.sum())
    total = float(np.log(np.maximum(r - E2, 1e-300)).sum()) - pos_total
    return np.float32(total / N)


def kernel(z_orig, z_augment):
    from concourse.bass_utils import run_bass_kernel_spmd

    nc = _get_nc()
    in_maps = make_in_maps(z_orig, z_augment)
    res = run_bass_kernel_spmd(nc, in_maps, core_ids=list(range(N_CORES)))
    return reduce_outputs(res.results)


# revision 30
# speedup vs baseline: 1.1013x; 1.1013x over previous
"""Trainium2 Bass kernel for nn_ContrastiveLoss2 (SimCLR NT-Xent loss).

Math (matches the jax reference):
    z  = concat([z_augment, z_orig])                       # [N=8192, D=256]
    zn = z / max(||z||, eps)                               # row L2 normalize
    S  = zn @ zn.T                                         # cosine sim [N, N]
    loss_i = -2 S[i, i+-B] + log( sum_{j != i} exp(2 S[i,j]) )
    out = mean_i loss_i                                    # tau = 0.5

Identity: denominator_i = sum_j exp(2 S_ij) - e^2 (S_ii == 1).

SYMMETRIC distribution (v2): S is symmetric, so each unordered pair is
computed once fleet-wide.  Core c (rows rolled so its own 1024 rows sit
at [0:1024)) computes the S block [5120 j-rows x 1024 own columns]:
  - j-tiles 0..31  (cores c..c+3):   full weight
  - j-tiles 32..39 (core c+4):       half weight, exp(2S - ln2); the
    partner core c+4 computes the transposed block also at half weight,
    so every pair still sums to 1.
Per-core outputs (denominators are assembled on the HOST):
  - csrow [2, 512]: column sums of exp over the block's 5120 j-rows
    (PE fp8 ones-matmul into PSUM) -> partial denominators for the
    core's own 1024 rows.  Row 0 holds cols 0:512, row 1 cols 512:1024.
  - rs [128, 32]: free-dim row sums of the exp tiles 8..39 -> partial
    denominators for rows owned by cores c+1..c+4.  ACT tiles get these
    free via the activation accumulator; Schraudolph tiles use a
    tensor_scalar+accum on DVE/Pool over the fp8 exp tile.
  - pose [128, 8]: the positive entries, read from the exp tiles
    (tiles 32..39 diagonal) by a Pool masked multiply+accum; the host
    inverts exp (or the Schraudolph bit pattern) to recover 2 S_pos.
Host: r_i = own colsum + 4 partner rowsums; loss = (sum ln(r_i - e^2)
- sum 2 S_pos) / N.

Engines: exp tiles split ACT (activation Exp, per-partition scale
inv_j) / DVE (Schraudolph: int8 = S*a_j + b bit-pattern IS fp8 exp);
norm prep: own rows via DVE tensor_tensor_reduce, j-rows via Pool
square + add-tree; PE: S matmuls + fp8 DoubleRow colsum accumulation.
"""

import sys

import numpy as np

try:
    import concourse  # noqa: F401
except ImportError:  # pragma: no cover
    sys.path.insert(0, "/opt/trn_rl_repo")

N_CORES = 8
N = 8192          # total rows (2B)
D = 256           # feature dim
B = 4096          # batch (positive offset)
P = 128           # SBUF partitions
NT = 40           # j-tiles per core (5/8 of 64)
NJ = NT * P       # 5120 j rows per core
RPC = 1024        # own columns per core
NI = RPC // P     # 8 own col-tiles
TAU = 0.5
E2 = float(np.exp(2.0))
LN2 = float(np.log(2.0))
A_EXP = 8.0 / LN2      # rhs carries the factor 2 -> a = inv * 8/ln2
SIGMA = 0.0435
# real-HW fp32->int8 convert rounds to nearest (the simulator truncates);
# calibrate for hardware, the graded correctness path
B_EXP = 56.0 - 8.0 * SIGMA

# per-tile exp engine assignment: A(CT) / D(VE Schraudolph).  Alternate
# from tile 0 so both engines fill immediately; extra A's spread mid/late
# to balance ACT ~22 tiles vs DVE ~18 + its fold/pos/rhs extras.
ASSIGN = (['A', 'D'] * 4
          + ['A', 'D', 'A', 'D', 'A', 'D', 'A', 'A'] * 2
          + ['A', 'D'] * 6 + ['A', 'D', 'A', 'A'])

# Pool prep chunks over j-tiles 8..39 (own tiles 0..7 use DVE ttr)
CHUNKS = [(8, 16), (16, 24), (24, 32), (32, 40)]


def _kernel_body(ctx, tc, csrow_ap, rs_ap, pose_ap, zn_ap, zt_ap):
    from concourse import mybir
    from concourse.masks import make_identity

    nc = tc.nc
    f32 = mybir.dt.float32
    bf16 = mybir.dt.bfloat16
    fp8 = mybir.dt.float8e4
    i8 = mybir.dt.int8
    Fn = mybir.ActivationFunctionType
    Op = mybir.AluOpType
    DR = mybir.MatmulPerfMode.DoubleRow

    p_const = ctx.enter_context(tc.tile_pool(name="const", bufs=1))
    p_z = ctx.enter_context(tc.tile_pool(name="z", bufs=1))
    p_sq = ctx.enter_context(tc.tile_pool(name="sq", bufs=1))
    p_tree = ctx.enter_context(tc.tile_pool(name="tree", bufs=1))
    p_stats = ctx.enter_context(tc.tile_pool(name="stats", bufs=1))
    p_ex = ctx.enter_context(tc.tile_pool(name="ex", bufs=6))
    p_junk = ctx.enter_context(tc.tile_pool(name="junk", bufs=2))
    p_s = ctx.enter_context(tc.tile_pool(name="s", bufs=3, space="PSUM"))
    p_cs = ctx.enter_context(tc.tile_pool(name="cs", bufs=1, space="PSUM"))

    znat = p_z.tile([P, NT, D], bf16, tag="znat", name="znat")
    zT = p_z.tile([P, 2, NJ], fp8, tag="zT", name="zT")
    sq = p_sq.tile([P, 8, D], bf16, tag="sq")
    # tree levels for Pool chunks: widths 128 ... 2 bf16, final add -> f32
    tl = [p_tree.tile([P, 8, D // (2 << k)], bf16, tag=f"tl{k}", name=f"tl{k}")
          for k in range(7)]
    sqj = p_sq.tile([P, D], bf16, tag="sqj")  # ttr junk out (own prep)
    ss = p_stats.tile([P, NT], f32, tag="ss")
    lns = p_stats.tile([P, NT], f32, tag="lns")
    inv = p_stats.tile([P, NT], f32, tag="inv")
    a_col = p_stats.tile([P, NT], f32, tag="a_col")
    inv2own = p_stats.tile([P, NI], f32, tag="inv2own")
    ln2_c = p_const.tile([P, 1], f32, tag="ln2c")
    nln2_c = p_const.tile([P, 1], f32, tag="nln2c")
    masked = p_stats.tile([P, RPC], bf16, tag="masked")
    rhs = p_z.tile([P, 2, RPC], fp8, tag="rhs", name="rhs")
    ones_bf = p_const.tile([P, P], bf16, tag="onesbf")
    ones = p_const.tile([P, 2, P], fp8, tag="ones")
    ident = p_const.tile([P, P], bf16, tag="ident")
    rs = p_stats.tile([P, 32], f32, tag="rs")
    posE = p_stats.tile([P, NI], f32, tag="posE")
    csrow = p_stats.tile([1, RPC], f32, tag="csrow")

    from concourse import library_config
    nc.gpsimd.load_library(library_config.proxy)

    # input DMAs, spread across queues; zT0 on ACT, zT1 on DVE so the SP
    # queue's serial issue stream starts with the prep-critical zn pieces
    def load_zn(eng, t0, t1):
        eng.dma_start(out=znat[:, t0:t1, :], in_=zn_ap[:, t0 * D:t1 * D]
                      .rearrange("p (t c) -> p t c", c=D))

    def load_zt(eng, k):
        eng.dma_start(out=zT[:, :, k * 1024:(k + 1) * 1024],
                      in_=zt_ap[:, :, k * 1024:(k + 1) * 1024]
                      .rearrange("h p j -> p h j"))

    load_zt(nc.scalar, 0)
    load_zn(nc.sync, 0, 4)
    load_zn(nc.sync, 4, 8)
    load_zn(nc.sync, 8, 16)
    load_zt(nc.sync, 1)
    load_zn(nc.sync, 16, 24)
    load_zt(nc.sync, 2)
    load_zn(nc.sync, 24, 32)
    load_zt(nc.sync, 3)
    load_zn(nc.sync, 32, 40)
    load_zt(nc.sync, 4)

    # consts (DVE memsets are cheap; ident is built on Pool)
    nc.vector.memset(ones_bf[:], 1.0)
    nc.vector.memset(ones[:], 1.0)
    nc.vector.memset(ln2_c[:], LN2)
    nc.vector.memset(nln2_c[:], -LN2)
    make_identity(nc, ident[:])
    junkw = p_const.tile([P, 512], bf16, tag="junkw")
    nc.gpsimd.memset(junkw[:], 0.0)

    # preload the Ln/Exp activation table off the critical path
    warm = p_const.tile([1, 1], f32, tag="warm")
    warm_o = p_const.tile([1, 1], f32, tag="warmo")
    nc.gpsimd.memset(warm[:], 1.0)
    nc.scalar.activation(warm_o[:], warm[:], Fn.Ln)

    cs = p_cs.tile([P, RPC], f32)

    # PE p-state warm-up: dummy matmuls into the (not yet used) cs banks
    # keep the Tensor engine continuously busy through the fill so the
    # first real matmuls run at full clock instead of 0.65 GHz
    for _ in range(10):
        nc.tensor.matmul(cs[:, 0:512], lhsT=ones_bf[:], rhs=junkw[:],
                         start=True, stop=True)

    # --- own-row prep: all 8 tiles on DVE (fused square+reduce via
    # stt+accum, the HW-safe tensor_tensor_reduce replacement), the ACT
    # ln/exp chain per 4-tile half overlapped with the second stt half ---
    for t in range(0, 8):
        nc.vector.scalar_tensor_tensor(
            sqj[:], znat[:, t, :], 1.0, znat[:, t, :],
            op0=Op.mult, op1=Op.mult, accum_out=ss[:, t:t + 1])

    # single chain with true dependencies so the scheduler cannot
    # interleave it into (and thereby stretch) the stt stream
    nc.scalar.activation(lns[:, 0:8], ss[:, 0:8], Fn.Ln)
    nc.scalar.activation(inv[:, 0:8], lns[:, 0:8], Fn.Exp, scale=-0.5)
    # 2/||z|| for the own rows (rhs carries the factor 2)
    nc.scalar.activation(inv2own[:], lns[:, 0:8], Fn.Exp,
                         scale=-0.5, bias=ln2_c[:])
    # replicate inv2own across partitions: identity-mask (broadcast
    # views, DVE 2x) + bf16 ones-matmul column sums into the cs PSUM
    # banks (free until the first colsum accumulation)
    nc.vector.tensor_tensor(
        masked[:].rearrange("p (t q) -> p t q", q=P),
        inv2own[:].rearrange("p (t o) -> p t o", o=1)
        .broadcast_to((P, NI, P)),
        ident[:].rearrange("p (o q) -> p o q", o=1)
        .broadcast_to((P, NI, P)),
        op=Op.mult)
    for h in range(2):
        nc.tensor.matmul(cs[:, h * 512:(h + 1) * 512], lhsT=ones_bf[:],
                         rhs=masked[:, h * 512:(h + 1) * 512],
                         start=True, stop=True)
        nc.vector.tensor_tensor(
            rhs[:, :, h * 512:(h + 1) * 512],
            zT[:, :, h * 512:(h + 1) * 512],
            cs[:, h * 512:(h + 1) * 512].rearrange("p (o c) -> p o c", o=1)
            .broadcast_to((P, 2, 512)),
            op=Op.mult)

    # --- Pool prep for j-tile chunks 8..39: square + binary add-tree.
    # All chunks share the same sq/tl scratch slots, so WAR/WAW deps keep
    # the chunks strictly in order (the scheduler otherwise hoists a later
    # chunk's big square in front of an earlier chunk's tree tail). ---
    def chunk_sq(t0, t1):
        nc.gpsimd.tensor_tensor(sq[:, 0:t1 - t0, :], znat[:, t0:t1, :],
                                znat[:, t0:t1, :], op=Op.mult)
        src = sq[:, 0:t1 - t0, :].rearrange(
            "p t (two c) -> p t two c", two=2)
        nc.gpsimd.tensor_tensor(tl[0][:, 0:t1 - t0, :], src[:, :, 0, :],
                                src[:, :, 1, :], op=Op.add)

    def chunk_tree(t0, t1, k0, k1):
        for k in range(k0, k1):
            s2 = tl[k][:, 0:t1 - t0, :].rearrange(
                "p t (two c) -> p t two c", two=2)
            nc.gpsimd.tensor_tensor(tl[k + 1][:, 0:t1 - t0, :],
                                    s2[:, :, 0, :], s2[:, :, 1, :], op=Op.add)

    def chunk_fin(t0, t1):
        s2 = tl[6][:, 0:t1 - t0, :]
        nc.gpsimd.tensor_tensor(
            ss[:, t0:t1].rearrange("p (t o) -> p t o", o=1),
            s2[:, :, 0:1], s2[:, :, 1:2], op=Op.add)
        nc.scalar.activation(lns[:, t0:t1], ss[:, t0:t1], Fn.Ln)
        nc.scalar.activation(inv[:, t0:t1], lns[:, t0:t1], Fn.Exp, scale=-0.5)
        nc.vector.tensor_scalar(a_col[:, t0:t1], inv[:, t0:t1], A_EXP, None,
                                op0=Op.mult)

    # a_col for own tiles (DVE Schraudolph scale)
    nc.vector.tensor_scalar(a_col[:, 0:8], inv[:, 0:8], A_EXP, None,
                            op0=Op.mult)

    ex_state = {}
    pend_cs = []

    def do_tile(t):
        u, slot = divmod(t, 2)
        if slot == 0:
            ex = p_ex.tile([P, 2, RPC], fp8, tag="ex", name="ex")
            ex_state['ex'] = ex
        else:
            ex = ex_state['ex']
        s_ps = p_s.tile([P, RPC], f32, tag="s", name="s_ps")
        for c in range(2):
            nc.tensor.matmul(
                s_ps[:, c * 512:(c + 1) * 512],
                lhsT=zT[:, :, t * P:(t + 1) * P],
                rhs=rhs[:, :, c * 512:(c + 1) * 512],
                start=True, stop=True, perf_mode=DR)
        half = t >= 32
        if ASSIGN[t] == 'A':
            kw = {}
            if t >= 8:
                kw['accum_out'] = rs[:, t - 8:t - 7]
            if half:
                kw['bias'] = nln2_c[:]
            nc.scalar.activation(ex[:, slot, :], s_ps[:], Fn.Exp,
                                 scale=inv[:, t:t + 1], **kw)
        else:
            b = B_EXP - (8.0 if half else 0.0)
            nc.vector.tensor_scalar(ex[:, slot, :].bitcast(i8), s_ps[:],
                                    a_col[:, t:t + 1], b,
                                    op0=Op.mult, op1=Op.add)
            if t >= 8:
                # rowsum: Pool folds the fp8 tile 1024 -> 64 with 4 TT
                # adds (no Pool tensor_scalar on real HW); DVE finishes
                # with a 2x tensor_scalar + accumulate
                exs = ex[:, slot, :]
                fold = p_junk.tile([P, 960], bf16, tag="fold", name="fold")
                nc.gpsimd.tensor_tensor(fold[:, 0:512], exs[0:P, 0:512],
                                        exs[0:P, 512:1024], op=Op.add)
                nc.gpsimd.tensor_tensor(fold[:, 512:768], fold[:, 0:256],
                                        fold[:, 256:512], op=Op.add)
                nc.gpsimd.tensor_tensor(fold[:, 768:896], fold[:, 512:640],
                                        fold[:, 640:768], op=Op.add)
                nc.gpsimd.tensor_tensor(fold[:, 896:960], fold[:, 768:832],
                                        fold[:, 832:896], op=Op.add)
                junk = p_junk.tile([P, 64], bf16, tag="junk", name="junk")
                nc.vector.tensor_scalar(junk[:], fold[:, 896:960], 1.0, None,
                                        op0=Op.mult, op1=Op.add,
                                        accum_out=rs[:, t - 8:t - 7])
        if half:
            # positive entries: diagonal of col-block (t-32) of the exp
            # tile, via DVE masked multiply + accumulate (all-SBUF 2x)
            k = t - 32
            junkm = p_junk.tile([P, P], bf16, tag="junkm", name="junkm")
            nc.vector.scalar_tensor_tensor(
                junkm[:], ex[:, slot, k * P:(k + 1) * P], 1.0, ident[:],
                op0=Op.mult, op1=Op.mult, accum_out=posE[:, k:k + 1])
        if slot == 1:
            pend_cs.append((u, ex))
        # defer colsum matmuls so a lagging exp pair can't stall the S
        # matmuls behind it in PE's in-order queue (taper the defer near
        # the end so the final flush isn't a burst after the last exp)
        defer = 7 if t < 32 else 3
        while pend_cs and (pend_cs[0][0] * 2 + defer <= t or t == NT - 1):
            uu, exx = pend_cs.pop(0)
            for c in range(2):
                nc.tensor.matmul(
                    cs[:, c * 512:(c + 1) * 512],
                    lhsT=ones[:], rhs=exx[:, :, c * 512:(c + 1) * 512],
                    start=(uu == 0), stop=(uu == NT // 2 - 1), perf_mode=DR)

    # interleave Pool chunk prep into the tile stream so chunk c's ops
    # don't block the Pool rowsum/pos ops of earlier tiles
    PREP_AT = {
        0: lambda: chunk_sq(8, 16),
        1: lambda: chunk_tree(8, 16, 0, 3),
        2: lambda: chunk_tree(8, 16, 3, 6),
        3: lambda: chunk_fin(8, 16),
        4: lambda: chunk_sq(16, 24),
        6: lambda: chunk_tree(16, 24, 0, 3),
        8: lambda: chunk_tree(16, 24, 3, 6),
        10: lambda: chunk_fin(16, 24),
        12: lambda: chunk_sq(24, 32),
        14: lambda: chunk_tree(24, 32, 0, 3),
        16: lambda: chunk_tree(24, 32, 3, 6),
        18: lambda: chunk_fin(24, 32),
        20: lambda: chunk_sq(32, 40),
        22: lambda: chunk_tree(32, 40, 0, 3),
        24: lambda: chunk_tree(32, 40, 3, 6),
        26: lambda: chunk_fin(32, 40),
    }
    for t in range(NT):
        do_tile(t)
        if t in PREP_AT:
            PREP_AT[t]()

    # tail: cs row readout split DVE/ACT (the 128 cs rows are identical),
    # then the two csrow halves DMA'd from separate queues in parallel
    nc.vector.tensor_scalar(csrow[:, 0:512], cs[0:1, 0:512], 1.0, None,
                            op0=Op.mult)
    nc.scalar.activation(csrow[:, 512:1024], cs[0:1, 512:1024], Fn.Copy)
    nc.sync.dma_start(out=pose_ap, in_=posE[:])
    nc.sync.dma_start(out=csrow_ap[:, 0:512], in_=csrow[:, 0:512])
    nc.scalar.dma_start(out=csrow_ap[:, 512:1024], in_=csrow[:, 512:1024])
    nc.sync.dma_start(out=rs_ap, in_=rs[:])


def build_nc():
    """Build (once) the Bass module shared by all 8 cores."""
    from contextlib import ExitStack

    from concourse import bacc, mybir
    import concourse.tile as tile

    nc = bacc.Bacc("TRN2", target_bir_lowering=False, debug=False)
    fp8 = mybir.dt.float8e4
    zn = nc.dram_tensor("zn", [P, NT * D], mybir.dt.bfloat16,
                        kind="ExternalInput").ap()
    zt = nc.dram_tensor("zt", [2, P, NJ], fp8, kind="ExternalInput").ap()
    csrow = nc.dram_tensor("csrow", [1, RPC], mybir.dt.float32,
                           kind="ExternalOutput").ap()
    rs = nc.dram_tensor("rs", [P, 32], mybir.dt.float32,
                        kind="ExternalOutput").ap()
    pose = nc.dram_tensor("pose", [P, NI], mybir.dt.float32,
                          kind="ExternalOutput").ap()
    with tile.TileContext(nc) as tc:
        with ExitStack() as ctx:
            _kernel_body(ctx, tc, csrow, rs, pose, zn, zt)
    return nc


_NC = None


def _get_nc(finalized=True):
    global _NC
    if _NC is None:
        _NC = build_nc()
    if finalized and not _NC.is_finalized():
        _NC.finalize()
    return _NC


def make_in_maps(z_orig, z_augment):
    from concourse import mybir

    f8np = mybir.dt.np(mybir.dt.float8e4)
    bfnp = mybir.dt.np(mybir.dt.bfloat16)
    z = np.ascontiguousarray(
        np.concatenate([np.asarray(z_augment, dtype=np.float32),
                        np.asarray(z_orig, dtype=np.float32)], axis=0))
    maps = []
    for c in range(N_CORES):
        zr = np.roll(z, -RPC * c, axis=0)[:NJ]
        zf8 = zr.astype(f8np)
        zbf = zr.astype(bfnp)
        # natural, pre-swizzled: zn[p, t*256 + c] = z[t*128+p, c]
        znat = np.ascontiguousarray(
            zbf.reshape(NT, P, D).transpose(1, 0, 2).reshape(P, NT * D))
        # transposed: zt[h, p, j] = z[j, 128h + p]
        zt = np.ascontiguousarray(zf8.T.reshape(2, P, NJ))
        maps.append({"zn": znat, "zt": zt})
    return maps


def reduce_outputs(results):
    """Host assembly: denominators from colsum + rowsums, ln, positives."""
    r = np.zeros(N, dtype=np.float64)
    pos_total = 0.0
    for c, res in enumerate(results):
        base = RPC * c
        cs = np.asarray(res["csrow"], dtype=np.float64).reshape(RPC)
        idx = (base + np.arange(RPC)) % N
        r[idx] += cs
        rsv = np.asarray(res["rs"], dtype=np.float64)  # [128, 32]
        for t in range(8, NT):
            jdx = (base + P * t + np.arange(P)) % N
            r[jdx] += rsv[:, t - 8]
        # positives: decode the exp-tile values back to 2 S_cos
        pe = np.asarray(res["pose"], dtype=np.float32)  # [128, 8]
        for k in range(NI):
            t = 32 + k
            v = pe[:, k].astype(np.float64)
            if ASSIGN[t] == 'A':
                # v = fp8(exp(2S - ln2)) -> 2S = ln(2 v)
                twos = np.log(np.maximum(2.0 * v, 1e-30))
            else:
                # v = fp8-bit-pattern Schraudolph: bits = 2S*8/ln2 + B - 8
                from concourse import mybir
                f8np = mybir.dt.np(mybir.dt.float8e4)
                bits = pe[:, k].astype(f8np).view(np.uint8).astype(np.float64)
                twos = (bits - (B_EXP - 8.0)) * LN2 / 8.0
            pos_total += float(twos.sum())
    total = float(np.log(np.maximum(r - E2, 1e-300)).sum()) - pos_total
    return np.float32(total / N)


def kernel(z_orig, z_augment):
    from concourse.bass_utils import run_bass_kernel_spmd

    nc = _get_nc()
    in_maps = make_in_maps(z_orig, z_augment)
    res = run_bass_kernel_spmd(nc, in_maps, core_ids=list(range(N_CORES)))
    return reduce_outputs(res.results)


# revision 34
# speedup vs baseline: 1.1543x; 1.0481x over previous
"""Trainium2 Bass kernel for nn_ContrastiveLoss2 (SimCLR NT-Xent loss).

Math (matches the jax reference):
    z  = concat([z_augment, z_orig])                       # [N=8192, D=256]
    zn = z / max(||z||, eps)                               # row L2 normalize
    S  = zn @ zn.T                                         # cosine sim [N, N]
    loss_i = -2 S[i, i+-B] + log( sum_{j != i} exp(2 S[i,j]) )
    out = mean_i loss_i                                    # tau = 0.5

Identity: denominator_i = sum_j exp(2 S_ij) - e^2 (S_ii == 1).

SYMMETRIC distribution (v2): S is symmetric, so each unordered pair is
computed once fleet-wide.  Core c (rows rolled so its own 1024 rows sit
at [0:1024)) computes the S block [5120 j-rows x 1024 own columns]:
  - j-tiles 0..31  (cores c..c+3):   full weight
  - j-tiles 32..39 (core c+4):       half weight, exp(2S - ln2); the
    partner core c+4 computes the transposed block also at half weight,
    so every pair still sums to 1.
Per-core outputs (denominators are assembled on the HOST):
  - csrow [2, 512]: column sums of exp over the block's 5120 j-rows
    (PE fp8 ones-matmul into PSUM) -> partial denominators for the
    core's own 1024 rows.  Row 0 holds cols 0:512, row 1 cols 512:1024.
  - rs [128, 32]: free-dim row sums of the exp tiles 8..39 -> partial
    denominators for rows owned by cores c+1..c+4.  ACT tiles get these
    free via the activation accumulator; Schraudolph tiles use a
    tensor_scalar+accum on DVE/Pool over the fp8 exp tile.
  - pose [128, 8]: the positive entries, read from the exp tiles
    (tiles 32..39 diagonal) by a Pool masked multiply+accum; the host
    inverts exp (or the Schraudolph bit pattern) to recover 2 S_pos.
Host: r_i = own colsum + 4 partner rowsums; loss = (sum ln(r_i - e^2)
- sum 2 S_pos) / N.

Engines: exp tiles split ACT (activation Exp, per-partition scale
inv_j) / DVE (Schraudolph: int8 = S*a_j + b bit-pattern IS fp8 exp);
norm prep: own rows via DVE tensor_tensor_reduce, j-rows via Pool
square + add-tree; PE: S matmuls + fp8 DoubleRow colsum accumulation.
"""

import sys

import numpy as np

try:
    import concourse  # noqa: F401
except ImportError:  # pragma: no cover
    sys.path.insert(0, "/opt/trn_rl_repo")

N_CORES = 8
N = 8192          # total rows (2B)
D = 256           # feature dim
B = 4096          # batch (positive offset)
P = 128           # SBUF partitions
NT = 40           # j-tiles per core (5/8 of 64)
NJ = NT * P       # 5120 j rows per core
RPC = 1024        # own columns per core
NI = RPC // P     # 8 own col-tiles
TAU = 0.5
E2 = float(np.exp(2.0))
LN2 = float(np.log(2.0))
A_EXP = 8.0 / LN2      # rhs carries the factor 2 -> a = inv * 8/ln2
SIGMA = 0.0435
# real-HW fp32->int8 convert rounds to nearest (the simulator truncates);
# calibrate for hardware, the graded correctness path
B_EXP = 56.0 - 8.0 * SIGMA

# per-tile exp engine assignment: A(CT) / D(VE Schraudolph).  Tiles 0..2
# run as ACT half-tiles during the fill (c0 halves before rhs2 exists);
# the rest alternate D-first, extra A's late where DVE has pos-extract
# work.  22 A vs 18 D balances ACT against DVE + its extras.
ASSIGN = (['A', 'A', 'A'] + ['D', 'A'] * 14 + ['D']
          + ['A', 'D', 'A', 'A', 'A', 'D', 'A', 'D'])

# Pool prep chunks over j-tiles 8..39 (own tiles 0..7 use DVE ttr)
CHUNKS = [(8, 16), (16, 24), (24, 32), (32, 40)]


def _kernel_body(ctx, tc, csrow_ap, rs_ap, pose_ap, zn_ap, zt_ap):
    from concourse import mybir
    from concourse.masks import make_identity

    nc = tc.nc
    f32 = mybir.dt.float32
    bf16 = mybir.dt.bfloat16
    fp8 = mybir.dt.float8e4
    i8 = mybir.dt.int8
    Fn = mybir.ActivationFunctionType
    Op = mybir.AluOpType
    DR = mybir.MatmulPerfMode.DoubleRow

    p_const = ctx.enter_context(tc.tile_pool(name="const", bufs=1))
    p_z = ctx.enter_context(tc.tile_pool(name="z", bufs=1))
    p_sq = ctx.enter_context(tc.tile_pool(name="sq", bufs=1))
    p_tree = ctx.enter_context(tc.tile_pool(name="tree", bufs=1))
    p_stats = ctx.enter_context(tc.tile_pool(name="stats", bufs=1))
    p_ex = ctx.enter_context(tc.tile_pool(name="ex", bufs=6))
    p_junk = ctx.enter_context(tc.tile_pool(name="junk", bufs=2))
    p_s = ctx.enter_context(tc.tile_pool(name="s", bufs=3, space="PSUM"))
    p_cs = ctx.enter_context(tc.tile_pool(name="cs", bufs=1, space="PSUM"))

    znat = p_z.tile([P, NT, D], bf16, tag="znat", name="znat")
    zT = p_z.tile([P, 2, NJ], fp8, tag="zT", name="zT")
    sq = p_sq.tile([P, 8, D], bf16, tag="sq")
    # tree levels for Pool chunks: widths 128 ... 2 bf16, final add -> f32
    tl = [p_tree.tile([P, 8, D // (2 << k)], bf16, tag=f"tl{k}", name=f"tl{k}")
          for k in range(7)]
    sqj = p_sq.tile([P, D], bf16, tag="sqj")  # ttr junk out (own prep)
    ss = p_stats.tile([P, NT], f32, tag="ss")
    lns = p_stats.tile([P, NT], f32, tag="lns")
    inv = p_stats.tile([P, NT], f32, tag="inv")
    a_col = p_stats.tile([P, NT], f32, tag="a_col")
    inv2own = p_stats.tile([P, NI], f32, tag="inv2own")
    ln2_c = p_const.tile([P, 1], f32, tag="ln2c")
    nln2_c = p_const.tile([P, 1], f32, tag="nln2c")
    masked = p_stats.tile([P, RPC], bf16, tag="masked")
    rhs = p_z.tile([P, 2, RPC], fp8, tag="rhs", name="rhs")
    ones_bf = p_const.tile([P, P], bf16, tag="onesbf")
    ones = p_const.tile([P, 2, P], fp8, tag="ones")
    ident = p_const.tile([P, P], bf16, tag="ident")
    rs = p_stats.tile([P, 32], f32, tag="rs")
    posE = p_stats.tile([P, NI], f32, tag="posE")
    csrow = p_stats.tile([1, RPC], f32, tag="csrow")

    from concourse import library_config
    nc.gpsimd.load_library(library_config.proxy)

    # input DMAs, spread across queues; zT0 on ACT, zT1 on DVE so the SP
    # queue's serial issue stream starts with the prep-critical zn pieces
    def load_zn(eng, t0, t1):
        eng.dma_start(out=znat[:, t0:t1, :], in_=zn_ap[:, t0 * D:t1 * D]
                      .rearrange("p (t c) -> p t c", c=D))

    def load_zt(eng, k):
        eng.dma_start(out=zT[:, :, k * 1024:(k + 1) * 1024],
                      in_=zt_ap[:, :, k * 1024:(k + 1) * 1024]
                      .rearrange("h p j -> p h j"))

    load_zt(nc.scalar, 0)
    load_zn(nc.sync, 0, 4)
    load_zn(nc.sync, 4, 8)
    load_zn(nc.sync, 8, 16)
    load_zt(nc.sync, 1)
    load_zn(nc.sync, 16, 24)
    load_zt(nc.sync, 2)
    load_zn(nc.sync, 24, 32)
    load_zt(nc.sync, 3)
    load_zn(nc.sync, 32, 40)
    load_zt(nc.sync, 4)

    # consts (DVE memsets are cheap; ident is built on Pool)
    nc.vector.memset(ones_bf[:], 1.0)
    nc.vector.memset(ones[:], 1.0)
    nc.vector.memset(ln2_c[:], LN2)
    nc.vector.memset(nln2_c[:], -LN2)
    make_identity(nc, ident[:])
    junkw = p_const.tile([P, 512], bf16, tag="junkw")
    nc.gpsimd.memset(junkw[:], 0.0)

    # preload the Ln/Exp activation table off the critical path
    warm = p_const.tile([1, 1], f32, tag="warm")
    warm_o = p_const.tile([1, 1], f32, tag="warmo")
    nc.gpsimd.memset(warm[:], 1.0)
    nc.scalar.activation(warm_o[:], warm[:], Fn.Ln)

    cs = p_cs.tile([P, RPC], f32)

    # PE p-state warm-up: dummy matmuls into the (not yet used) cs banks
    # keep the Tensor engine continuously busy through the fill so the
    # first real matmuls run at full clock instead of 0.65 GHz
    for _ in range(10):
        nc.tensor.matmul(cs[:, 0:512], lhsT=ones_bf[:], rhs=junkw[:],
                         start=True, stop=True)

    # --- own-row prep: all 8 tiles on DVE (fused square+reduce via
    # stt+accum, the HW-safe tensor_tensor_reduce replacement), the ACT
    # ln/exp chain per 4-tile half overlapped with the second stt half ---
    for t in range(0, 8):
        nc.vector.scalar_tensor_tensor(
            sqj[:], znat[:, t, :], 1.0, znat[:, t, :],
            op0=Op.mult, op1=Op.mult, accum_out=ss[:, t:t + 1])

    # single chain with true dependencies so the scheduler cannot
    # interleave it into (and thereby stretch) the stt stream
    nc.scalar.activation(lns[:, 0:8], ss[:, 0:8], Fn.Ln)
    nc.scalar.activation(inv[:, 0:8], lns[:, 0:8], Fn.Exp, scale=-0.5)
    # 2/||z|| for the own rows (rhs carries the factor 2)
    nc.scalar.activation(inv2own[:], lns[:, 0:8], Fn.Exp,
                         scale=-0.5, bias=ln2_c[:])
    # replicate inv2own across partitions: identity-mask (broadcast
    # views, DVE 2x) + bf16 ones-matmul column sums into the cs PSUM
    # banks (free until the first colsum accumulation)
    nc.vector.tensor_tensor(
        masked[:].rearrange("p (t q) -> p t q", q=P),
        inv2own[:].rearrange("p (t o) -> p t o", o=1)
        .broadcast_to((P, NI, P)),
        ident[:].rearrange("p (o q) -> p o q", o=1)
        .broadcast_to((P, NI, P)),
        op=Op.mult)
    for h in range(2):
        nc.tensor.matmul(cs[:, h * 512:(h + 1) * 512], lhsT=ones_bf[:],
                         rhs=masked[:, h * 512:(h + 1) * 512],
                         start=True, stop=True)
        nc.vector.tensor_tensor(
            rhs[:, :, h * 512:(h + 1) * 512],
            zT[:, :, h * 512:(h + 1) * 512],
            cs[:, h * 512:(h + 1) * 512].rearrange("p (o c) -> p o c", o=1)
            .broadcast_to((P, 2, 512)),
            op=Op.mult)

    # --- Pool prep for j-tile chunks 8..39: square + binary add-tree.
    # All chunks share the same sq/tl scratch slots, so WAR/WAW deps keep
    # the chunks strictly in order (the scheduler otherwise hoists a later
    # chunk's big square in front of an earlier chunk's tree tail). ---
    def chunk_sq(t0, t1):
        nc.gpsimd.tensor_tensor(sq[:, 0:t1 - t0, :], znat[:, t0:t1, :],
                                znat[:, t0:t1, :], op=Op.mult)
        src = sq[:, 0:t1 - t0, :].rearrange(
            "p t (two c) -> p t two c", two=2)
        nc.gpsimd.tensor_tensor(tl[0][:, 0:t1 - t0, :], src[:, :, 0, :],
                                src[:, :, 1, :], op=Op.add)

    def chunk_tree(t0, t1, k0, k1):
        for k in range(k0, k1):
            s2 = tl[k][:, 0:t1 - t0, :].rearrange(
                "p t (two c) -> p t two c", two=2)
            nc.gpsimd.tensor_tensor(tl[k + 1][:, 0:t1 - t0, :],
                                    s2[:, :, 0, :], s2[:, :, 1, :], op=Op.add)

    def chunk_fin(t0, t1):
        s2 = tl[6][:, 0:t1 - t0, :]
        nc.gpsimd.tensor_tensor(
            ss[:, t0:t1].rearrange("p (t o) -> p t o", o=1),
            s2[:, :, 0:1], s2[:, :, 1:2], op=Op.add)
        nc.scalar.activation(lns[:, t0:t1], ss[:, t0:t1], Fn.Ln)
        nc.scalar.activation(inv[:, t0:t1], lns[:, t0:t1], Fn.Exp, scale=-0.5)
        nc.vector.tensor_scalar(a_col[:, t0:t1], inv[:, t0:t1], A_EXP, None,
                                op0=Op.mult)

    # a_col for own tiles (DVE Schraudolph scale)
    nc.vector.tensor_scalar(a_col[:, 0:8], inv[:, 0:8], A_EXP, None,
                            op0=Op.mult)

    ex_state = {}
    pend_cs = []

    # fill phase: tiles 0..2 as ACT half-tiles, each half in its own PSUM
    # pool allocation (PSUM deps are per-tile-semaphore, so a full-width
    # tile's first half could not be exp'd before the second matmul).
    # All c0 halves are allocated before the c1 halves so the 3-slot ring
    # frees in the right order, and c0 work only needs the first rhs half.
    ex01 = p_ex.tile([P, 2, RPC], fp8, tag="ex", name="ex")
    ex23 = p_ex.tile([P, 2, RPC], fp8, tag="ex", name="ex")
    ex_state['ex'] = ex23
    for c in range(2):
        for t in range(3):
            sh = p_s.tile([P, RPC], f32, tag="s", name="s_ps")
            nc.tensor.matmul(
                sh[:, 0:512], lhsT=zT[:, :, t * P:(t + 1) * P],
                rhs=rhs[:, :, c * 512:(c + 1) * 512],
                start=True, stop=True, perf_mode=DR)
            ext = ex01 if t < 2 else ex23
            nc.scalar.activation(ext[:, t % 2, c * 512:(c + 1) * 512],
                                 sh[:, 0:512], Fn.Exp, scale=inv[:, t:t + 1])
    pend_cs.append((0, ex01))

    def do_tile(t):
        u, slot = divmod(t, 2)
        if slot == 0:
            ex = p_ex.tile([P, 2, RPC], fp8, tag="ex", name="ex")
            ex_state['ex'] = ex
        else:
            ex = ex_state['ex']
        s_ps = p_s.tile([P, RPC], f32, tag="s", name="s_ps")
        for c in range(2):
            nc.tensor.matmul(
                s_ps[:, c * 512:(c + 1) * 512],
                lhsT=zT[:, :, t * P:(t + 1) * P],
                rhs=rhs[:, :, c * 512:(c + 1) * 512],
                start=True, stop=True, perf_mode=DR)
        half = t >= 32
        if ASSIGN[t] == 'A':
            kw = {}
            if t >= 8:
                kw['accum_out'] = rs[:, t - 8:t - 7]
            if half:
                kw['bias'] = nln2_c[:]
            nc.scalar.activation(ex[:, slot, :], s_ps[:], Fn.Exp,
                                 scale=inv[:, t:t + 1], **kw)
        else:
            b = B_EXP - (8.0 if half else 0.0)
            nc.vector.tensor_scalar(ex[:, slot, :].bitcast(i8), s_ps[:],
                                    a_col[:, t:t + 1], b,
                                    op0=Op.mult, op1=Op.add)
            if t >= 8:
                # rowsum: Pool folds the fp8 tile 1024 -> 64 with 4 TT
                # adds (no Pool tensor_scalar on real HW); DVE finishes
                # with a 2x tensor_scalar + accumulate
                exs = ex[:, slot, :]
                fold = p_junk.tile([P, 960], bf16, tag="fold", name="fold")
                nc.gpsimd.tensor_tensor(fold[:, 0:512], exs[0:P, 0:512],
                                        exs[0:P, 512:1024], op=Op.add)
                nc.gpsimd.tensor_tensor(fold[:, 512:768], fold[:, 0:256],
                                        fold[:, 256:512], op=Op.add)
                nc.gpsimd.tensor_tensor(fold[:, 768:896], fold[:, 512:640],
                                        fold[:, 640:768], op=Op.add)
                nc.gpsimd.tensor_tensor(fold[:, 896:960], fold[:, 768:832],
                                        fold[:, 832:896], op=Op.add)
                junk = p_junk.tile([P, 64], bf16, tag="junk", name="junk")
                nc.vector.tensor_scalar(junk[:], fold[:, 896:960], 1.0, None,
                                        op0=Op.mult, op1=Op.add,
                                        accum_out=rs[:, t - 8:t - 7])
        if half:
            # positive entries: diagonal of col-block (t-32) of the exp
            # tile, via DVE masked multiply + accumulate (all-SBUF 2x)
            k = t - 32
            junkm = p_junk.tile([P, P], bf16, tag="junkm", name="junkm")
            nc.vector.scalar_tensor_tensor(
                junkm[:], ex[:, slot, k * P:(k + 1) * P], 1.0, ident[:],
                op0=Op.mult, op1=Op.mult, accum_out=posE[:, k:k + 1])
        if slot == 1:
            pend_cs.append((u, ex))
        # defer colsum matmuls so a lagging exp pair can't stall the S
        # matmuls behind it in PE's in-order queue (taper the defer near
        # the end so the final flush isn't a burst after the last exp)
        defer = 7 if t < 32 else 3
        while pend_cs and (pend_cs[0][0] * 2 + defer <= t or t == NT - 1):
            uu, exx = pend_cs.pop(0)
            for c in range(2):
                nc.tensor.matmul(
                    cs[:, c * 512:(c + 1) * 512],
                    lhsT=ones[:], rhs=exx[:, :, c * 512:(c + 1) * 512],
                    start=(uu == 0), stop=(uu == NT // 2 - 1), perf_mode=DR)

    # interleave Pool chunk prep into the tile stream so chunk c's ops
    # don't block the Pool rowsum/pos ops of earlier tiles
    PREP_AT = {
        3: lambda: (chunk_sq(8, 16), chunk_tree(8, 16, 0, 3)),
        4: lambda: chunk_tree(8, 16, 3, 6),
        5: lambda: chunk_fin(8, 16),
        6: lambda: chunk_sq(16, 24),
        7: lambda: chunk_tree(16, 24, 0, 3),
        9: lambda: chunk_tree(16, 24, 3, 6),
        11: lambda: chunk_fin(16, 24),
        13: lambda: chunk_sq(24, 32),
        15: lambda: chunk_tree(24, 32, 0, 3),
        17: lambda: chunk_tree(24, 32, 3, 6),
        19: lambda: chunk_fin(24, 32),
        21: lambda: chunk_sq(32, 40),
        23: lambda: chunk_tree(32, 40, 0, 3),
        25: lambda: chunk_tree(32, 40, 3, 6),
        27: lambda: chunk_fin(32, 40),
    }
    for t in range(3, NT):
        do_tile(t)
        if t in PREP_AT:
            PREP_AT[t]()

    # tail: cs row readout split DVE/ACT (the 128 cs rows are identical),
    # then the two csrow halves DMA'd from separate queues in parallel
    nc.vector.tensor_scalar(csrow[:, 0:512], cs[0:1, 0:512], 1.0, None,
                            op0=Op.mult)
    nc.scalar.activation(csrow[:, 512:1024], cs[0:1, 512:1024], Fn.Copy)
    nc.sync.dma_start(out=pose_ap, in_=posE[:])
    nc.sync.dma_start(out=csrow_ap[:, 0:512], in_=csrow[:, 0:512])
    nc.scalar.dma_start(out=csrow_ap[:, 512:1024], in_=csrow[:, 512:1024])
    nc.sync.dma_start(out=rs_ap, in_=rs[:])


def build_nc():
    """Build (once) the Bass module shared by all 8 cores."""
    from contextlib import ExitStack

    from concourse import bacc, mybir
    import concourse.tile as tile

    nc = bacc.Bacc("TRN2", target_bir_lowering=False, debug=False)
    fp8 = mybir.dt.float8e4
    zn = nc.dram_tensor("zn", [P, NT * D], mybir.dt.bfloat16,
                        kind="ExternalInput").ap()
    zt = nc.dram_tensor("zt", [2, P, NJ], fp8, kind="ExternalInput").ap()
    csrow = nc.dram_tensor("csrow", [1, RPC], mybir.dt.float32,
                           kind="ExternalOutput").ap()
    rs = nc.dram_tensor("rs", [P, 32], mybir.dt.float32,
                        kind="ExternalOutput").ap()
    pose = nc.dram_tensor("pose", [P, NI], mybir.dt.float32,
                          kind="ExternalOutput").ap()
    with tile.TileContext(nc) as tc:
        with ExitStack() as ctx:
            _kernel_body(ctx, tc, csrow, rs, pose, zn, zt)
    return nc


_NC = None


def _get_nc(finalized=True):
    global _NC
    if _NC is None:
        _NC = build_nc()
    if finalized and not _NC.is_finalized():
        _NC.finalize()
    return _NC


def make_in_maps(z_orig, z_augment):
    from concourse import mybir

    f8np = mybir.dt.np(mybir.dt.float8e4)
    bfnp = mybir.dt.np(mybir.dt.bfloat16)
    z = np.ascontiguousarray(
        np.concatenate([np.asarray(z_augment, dtype=np.float32),
                        np.asarray(z_orig, dtype=np.float32)], axis=0))
    maps = []
    for c in range(N_CORES):
        zr = np.roll(z, -RPC * c, axis=0)[:NJ]
        zf8 = zr.astype(f8np)
        zbf = zr.astype(bfnp)
        # natural, pre-swizzled: zn[p, t*256 + c] = z[t*128+p, c]
        znat = np.ascontiguousarray(
            zbf.reshape(NT, P, D).transpose(1, 0, 2).reshape(P, NT * D))
        # transposed: zt[h, p, j] = z[j, 128h + p]
        zt = np.ascontiguousarray(zf8.T.reshape(2, P, NJ))
        maps.append({"zn": znat, "zt": zt})
    return maps


def reduce_outputs(results):
    """Host assembly: denominators from colsum + rowsums, ln, positives."""
    r = np.zeros(N, dtype=np.float64)
    pos_total = 0.0
    for c, res in enumerate(results):
        base = RPC * c
        cs = np.asarray(res["csrow"], dtype=np.float64).reshape(RPC)
        idx = (base + np.arange(RPC)) % N
        r[idx] += cs
        rsv = np.asarray(res["rs"], dtype=np.float64)  # [128, 32]
        for t in range(8, NT):
            jdx = (base + P * t + np.arange(P)) % N
            r[jdx] += rsv[:, t - 8]
        # positives: decode the exp-tile values back to 2 S_cos
        pe = np.asarray(res["pose"], dtype=np.float32)  # [128, 8]
        for k in range(NI):
            t = 32 + k
            v = pe[:, k].astype(np.float64)
            if ASSIGN[t] == 'A':
                # v = fp8(exp(2S - ln2)) -> 2S = ln(2 v)
                twos = np.log(np.maximum(2.0 * v, 1e-30))
            else:
                # v = fp8-bit-pattern Schraudolph: bits = 2S*8/ln2 + B - 8
                from concourse import mybir
                f8np = mybir.dt.np(mybir.dt.float8e4)
                bits = pe[:, k].astype(f8np).view(np.uint8).astype(np.float64)
                twos = (bits - (B_EXP - 8.0)) * LN2 / 8.0
            pos_total += float(twos.sum())
    total = float(np.log(np.maximum(r - E2, 1e-300)).sum()) - pos_total
    return np.float32(total / N)


def kernel(z_orig, z_augment):
    from concourse.bass_utils import run_bass_kernel_spmd

    nc = _get_nc()
    in_maps = make_in_maps(z_orig, z_augment)
    res = run_bass_kernel_spmd(nc, in_maps, core_ids=list(range(N_CORES)))
    return reduce_outputs(res.results)
